# revision 14
# baseline (speedup 1.0000x reference)
"""BiasedSelfAttention Trainium2 kernel, 8-core SPMD.

Reference computation (per batch b, head h):
    qkv = x @ W_attn + b_attn;  Q,K,V = split(qkv)
    S   = Q K^T / sqrt(hd)
    A   = softmax(S, axis=-1) + attn_B          (post-softmax additive bias)
    y   = A @ V

Sharding: 2 batches x 16 heads = 32 (b,h) pairs -> 4 heads/core,
core c handles batch c//4, heads [4*(c%4), 4*(c%4)+4).

Per-core kernel (natural [sq, sk] orientation):
  - QKV projection from host-pretransposed x^T with fp32r matmuls
    (Q^T/K^T packed [128, s] per head; V [s, 64] bf16, bias via K=1 matmul)
  - S tiles on PE (fp32r, N=512), exp+rowsum fused on ACT (accum_out)
  - A = expS * (1/rowsum) + B in one DVE scalar_tensor_tensor pass (bf16 out)
  - A^T via PE transpose (bf16, 1 cyc/row), PSUM->SBUF copies split DVE/ACT
  - y = sum_k A^T_chunk.T @ V_chunk accumulated in PSUM
"""

import numpy as np
import ml_dtypes


def _to_bf16(a):
    return a.astype(ml_dtypes.bfloat16)


import concourse.bass as bass
import concourse.mybir as mybir
import concourse.tile as tile
from concourse import bacc
from concourse.bass_utils import run_bass_kernel_spmd
from concourse.masks import make_identity

B, S, D = 2, 2048, 1024
H, HD = 16, 64
NCORES = 8
HPC = 4                 # heads per core
GD = HPC * HD           # 256 per-core output columns
KO = D // 128           # 8 contraction chunks for QKV
SQ = S // 128           # 16 seq chunks of 128
ST = S // 512           # 4 seq tiles of 512

fp32 = mybir.dt.float32
fp32r = mybir.dt.float32r
bf16 = mybir.dt.bfloat16

_CACHED_NC = None


def build_nc(repeat=1):
    """repeat>1 wraps the whole body in a hardware loop (for HW timing)."""
    nc = bacc.Bacc()

    xT = nc.declare_dram_parameter("xT", [D, S], fp32r, isOutput=False)
    # head-pair packed Q/K weights: [:, i, 0, :] = Q cols of heads (2i, 2i+1),
    # [:, i, 1, :] = K cols of heads (2i, 2i+1)
    wqk = nc.declare_dram_parameter("wqk", [D, 2, 2, 128], fp32r, isOutput=False)
    wv = nc.declare_dram_parameter("wv", [D, GD], fp32r, isOutput=False)
    bqk = nc.declare_dram_parameter("bqk", [128, 2, 2], fp32, isOutput=False)
    bv = nc.declare_dram_parameter("bv", [1, GD], bf16, isOutput=False)
    Bb = nc.declare_dram_parameter("Bb", [HPC, S, S], fp32, isOutput=False)
    y = nc.declare_dram_parameter("y", [S, GD], fp32, isOutput=True)

    import contextlib

    with tile.TileContext(nc) as tc:
        with (
            tc.For_i(0, repeat, 1) if repeat > 1 else contextlib.nullcontext(),
            tc.tile_pool(name="persist", bufs=1) as persist,
            tc.tile_pool(name="small", bufs=1) as small,
        ):
            # ---- persistent SBUF tensors ----
            # per head-pair: partitions 0:64 = head 2i, 64:128 = head 2i+1;
            # free dim: [:, 0, :] = Q^T rows, [:, 1, :] = K^T rows
            qk2 = [persist.tile([128, 2, S], fp32r, tag=f"qk2_{i}", name=f"qk2_{i}")
                   for i in range(HPC // 2)]
            v_sb = persist.tile([128, SQ, GD], bf16, tag="v_sb")
            ident = small.tile([128, 128], bf16, tag="ident")
            make_identity(nc, ident)
            bqk_sb = small.tile([128, 2, 2], fp32, tag="bqk_sb")
            nc.sync.dma_start(out=bqk_sb, in_=bqk[:, :])
            bv_sb = small.tile([1, GD], bf16, tag="bv_sb")
            nc.sync.dma_start(out=bv_sb, in_=bv[:, :])
            ones1 = small.tile([1, 128], bf16, tag="ones1")
            nc.vector.memset(ones1, 1.0)

            # ---- phase 1: QKV projection ----
            with (
                tc.tile_pool(name="p1sb", bufs=1) as p1sb,
                tc.tile_pool(name="p1ps", bufs=2, space="PSUM") as p1ps,
                tc.tile_pool(name="p1psv", bufs=2, space="PSUM") as p1psv,
            ):
                xts = p1sb.tile([128, KO, S], fp32r, tag="xts")
                nc.sync.dma_start(
                    out=xts, in_=xT.rearrange("(ko p) s -> p ko s", p=128)
                )
                wqk_sb = p1sb.tile([128, KO, 2, 2, 128], fp32r, tag="wqk_sb")
                nc.sync.dma_start(
                    out=wqk_sb, in_=wqk.rearrange("(ko p) i qk m -> p ko i qk m", p=128)
                )
                wv_sb = p1sb.tile([128, KO, GD], fp32r, tag="wv_sb")
                nc.sync.dma_start(
                    out=wv_sb, in_=wv.rearrange("(ko p) n -> p ko n", p=128)
                )

                # Q^T (resp K^T) of a head pair land on partitions 0:64 / 64:128
                for i in range(HPC // 2):
                    for qk in range(2):
                        for t in range(ST):
                            ps = p1ps.tile([128, 512], fp32, tag="ps_qk")
                            for ko in range(KO):
                                nc.tensor.matmul(
                                    ps,
                                    wqk_sb[:, ko, i, qk, :],
                                    xts[:, ko, t * 512:(t + 1) * 512],
                                    start=(ko == 0),
                                    stop=(ko == KO - 1),
                                )
                            # PSUM -> SBUF + per-partition bias (rounds to fp32r)
                            nc.scalar.activation(
                                qk2[i][:, qk, t * 512:(t + 1) * 512],
                                ps,
                                mybir.ActivationFunctionType.Identity,
                                bias=bqk_sb[:, i, qk:qk + 1],
                                scale=1.0,
                            )

                # V (all heads packed on free dim): [sk, GD] in bf16
                for kc in range(SQ):
                    psv = p1psv.tile([128, GD], fp32, tag="ps_v")
                    for ko in range(KO):
                        nc.tensor.matmul(
                            psv,
                            xts[:, ko, kc * 128:(kc + 1) * 128],
                            wv_sb[:, ko, :],
                            start=(ko == 0),
                            stop=False,
                        )
                    nc.tensor.matmul(psv, ones1, bv_sb, start=False, stop=True)
                    nc.vector.tensor_copy(v_sb[:, kc, :], psv)

            # ---- phase 2: attention ----
            with (
                tc.tile_pool(name="p2sb", bufs=2) as p2sb,
                tc.tile_pool(name="bpool", bufs=3) as bpool,
                tc.tile_pool(name="ypool", bufs=2) as ypool,
                tc.tile_pool(name="ps_s", bufs=1, space="PSUM") as psum_s,
                tc.tile_pool(name="ps_t", bufs=2, space="PSUM") as psum_t,
                tc.tile_pool(name="ps_y", bufs=2, space="PSUM") as psum_y,
            ):
                for q16 in range(SQ):
                    y_sb = ypool.tile([128, GD], fp32, tag="y_sb")
                    for h in range(HPC):
                        # bias tile for this (head, row-block): [128, S]
                        Bt = bpool.tile([128, S], fp32, tag="Bt")
                        nc.sync.dma_start(
                            out=Bt, in_=Bb[h, q16 * 128:(q16 + 1) * 128, :]
                        )

                        # scores S[sq, sk] for 128 rows x full S cols
                        pair, off = h // 2, 64 * (h % 2)
                        ps_s = psum_s.tile([128, S], fp32, tag="ps_s")
                        for t in range(ST):
                            nc.tensor.matmul(
                                ps_s[:, t * 512:(t + 1) * 512],
                                qk2[pair][off:off + 64, 0,
                                          q16 * 128:(q16 + 1) * 128],
                                qk2[pair][off:off + 64, 1,
                                          t * 512:(t + 1) * 512],
                                start=True,
                                stop=True,
                            )

                        # exp(S/8) with fused row-sums
                        expS = p2sb.tile([128, S], fp32, tag="expS")
                        sums = p2sb.tile([128, 1], fp32, tag="sums")
                        nc.scalar.activation(
                            expS,
                            ps_s,
                            mybir.ActivationFunctionType.Exp,
                            scale=0.125,
                            accum_out=sums,
                        )
                        recip = p2sb.tile([128, 1], fp32, tag="recip")
                        nc.vector.reciprocal(recip, sums)

                        # A = expS * recip + B   (bf16)
                        At = p2sb.tile([128, S], bf16, tag="At")
                        nc.vector.scalar_tensor_tensor(
                            At,
                            expS,
                            recip,
                            Bt,
                            op0=mybir.AluOpType.mult,
                            op1=mybir.AluOpType.add,
                        )

                        # transpose A in 128x128 blocks; copies split ACT/DVE
                        AT = p2sb.tile([128, SQ, 128], bf16, tag="AT")
                        for j in range(SQ // 4):
                            ps_tr = psum_t.tile([128, 4, 128], bf16, tag="ps_tr")
                            for jj in range(4):
                                kc = 4 * j + jj
                                nc.tensor.transpose(
                                    ps_tr[:, jj, :],
                                    At[:, kc * 128:(kc + 1) * 128],
                                    ident,
                                )
                            dst = AT[:, 4 * j:4 * j + 4, :]
                            if j % 2 == 0:
                                nc.vector.tensor_copy(dst, ps_tr)
                            else:
                                nc.scalar.copy(dst, ps_tr)

                        # y[sq, dv] = sum_kc AT[kc].T @ V[kc]
                        y_ps = psum_y.tile([128, HD], fp32, tag="y_ps")
                        for kc in range(SQ):
                            nc.tensor.matmul(
                                y_ps,
                                AT[:, kc, :],
                                v_sb[:, kc, h * HD:(h + 1) * HD],
                                start=(kc == 0),
                                stop=(kc == SQ - 1),
                            )
                        nc.vector.tensor_copy(y_sb[:, h * HD:(h + 1) * HD], y_ps)

                    nc.sync.dma_start(
                        out=y[q16 * 128:(q16 + 1) * 128, :], in_=y_sb
                    )

    nc.finalize()
    return nc


def _prep_core_inputs(x, attn_B, W_attn, b_attn, core):
    bi, g = core // 4, core % 4
    h0 = HPC * g
    xT = np.ascontiguousarray(x[bi].T)                       # [D, S]
    wqk = np.empty((D, 2, 2, 128), np.float32)
    bqk = np.empty((128, 2, 2), np.float32)
    for i in range(HPC // 2):
        for j in range(2):                                   # head within pair
            gh = h0 + 2 * i + j
            sl = slice(64 * j, 64 * j + 64)
            wqk[:, i, 0, sl] = W_attn[:, gh * 64:(gh + 1) * 64]
            wqk[:, i, 1, sl] = W_attn[:, D + gh * 64:D + (gh + 1) * 64]
            bqk[sl, i, 0] = b_attn[gh * 64:(gh + 1) * 64]
            bqk[sl, i, 1] = b_attn[D + gh * 64:D + (gh + 1) * 64]
    wv = np.ascontiguousarray(W_attn[:, 2 * D + g * GD:2 * D + (g + 1) * GD])
    bv = np.ascontiguousarray(b_attn[2 * D + g * GD:2 * D + (g + 1) * GD])
    Bb = np.ascontiguousarray(attn_B[bi, h0:h0 + HPC])
    return {
        "xT": xT, "wqk": wqk, "wv": wv, "bqk": bqk,
        "bv": bv.reshape(1, GD).astype(np.dtype("bfloat16")) if hasattr(np, "bfloat16") else _to_bf16(bv.reshape(1, GD)), "Bb": Bb,
    }


def kernel(x, attn_B, W_attn, b_attn):
    global _CACHED_NC
    x = np.asarray(x, np.float32)
    attn_B = np.asarray(attn_B, np.float32)
    W_attn = np.asarray(W_attn, np.float32)
    b_attn = np.asarray(b_attn, np.float32)

    if _CACHED_NC is None:
        _CACHED_NC = build_nc()
    nc = _CACHED_NC

    in_maps = [
        _prep_core_inputs(x, attn_B, W_attn, b_attn, c) for c in range(NCORES)
    ]
    res = run_bass_kernel_spmd(nc, in_maps, core_ids=list(range(NCORES)))

    out = np.empty((B, S, D), np.float32)
    for c in range(NCORES):
        bi, g = c // 4, c % 4
        out[bi, :, g * GD:(g + 1) * GD] = res.results[c]["y"]
    return out
